# revision 1
# baseline (speedup 1.0000x reference)
"""Per-sample dynamic 3x3 conv (DCConv2d) on 8 Trainium2 NeuronCores.

Strategy: pure data parallel. Each core gets B_LOCAL=16 samples. Inside a
core, samples are processed in groups of 4, packed onto the PE array as four
diagonal 32x32 tiles (tile_position=(32s,32s)) so the four per-sample
matmuls run concurrently on different sub-arrays. The 3x3 conv is 9
PSUM-accumulated matmuls (one per tap) over zero-padded images resident in
SBUF as [128 partitions = (sample, channel), 130, 130]. Matmuls run as
float32r (1 cycle/row at N=512). Per-sample weights (inputs_se @ bank) are
generated on-device from the replicated weight bank via ACT per-partition
scale + DVE adds. Bias is fused into the PSUM->SBUF epilogue on ACT.
"""

import numpy as np

import concourse.bass as bass
import concourse.mybir as mybir
import concourse.tile as tile
from concourse.bass_utils import run_bass_kernel_spmd

N_CORES = 8
B, C, H, W = 128, 32, 128, 128
O = 32
NUM = 8
KK = 3
B_LOCAL = B // N_CORES          # 16
GROUP = 4                       # samples packed per PE pass
N_GROUPS = B_LOCAL // GROUP     # 4
HP, WP = H + 2, W + 2           # zero-padded image dims
ROWS_PER_CHUNK = 4              # output rows per matmul chunk (N = 4*128 = 512)
N_CHUNKS = H // ROWS_PER_CHUNK  # 32
NTAPS = KK * KK                 # 9

F32 = mybir.dt.float32
F32R = mybir.dt.float32r
BF16 = mybir.dt.bfloat16


def _split_multiwait_insts(nc):
    """This walrus build encodes at most one sync-wait per instruction; Tile's
    tail drain carries one wait per hardware proc used. Split the extras into
    single-wait NOPs on the same engine, inserted just before."""
    for f in nc.m.functions:
        for blk in f.blocks:
            new_list = []
            changed = False
            for inst in blk.instructions:
                si = inst.sync_info
                if si is not None and len(si.on_wait) > 1:
                    waits = list(si.on_wait)
                    for j, w in enumerate(waits[:-1]):
                        new_list.append(
                            mybir.InstNoOp(
                                name=f"{inst.name}-ws-{j}",
                                engine=inst.engine,
                                ins=[],
                                outs=[],
                                sync_info=mybir.SyncInfo(on_wait=[w], on_update=[]),
                            )
                        )
                    inst.sync_info = mybir.SyncInfo(
                        on_wait=[waits[-1]], on_update=list(si.on_update)
                    )
                    changed = True
                new_list.append(inst)
            if changed:
                blk.instructions = new_list


def build_program(b_local=B_LOCAL, split_waits=True, reps=1,
                  abl_no_store=False, abl_no_xdma=False, abl_no_mm=False):
    n_groups = b_local // GROUP
    nc = bass.Bass(
        "TRN2",
        target_bir_lowering=False,
        debug=False,
        num_devices=N_CORES,
        enable_partition_id=False,
    )
    x_d = nc.dram_tensor("x", [b_local, C, H, W], F32, kind="ExternalInput").ap()
    # wb: host-permuted weight bank, wb[i, n, t*32+o] = weight[o*288+i*9+t, n]
    wb_d = nc.dram_tensor("wb", [C, NUM, NTAPS * O], F32, kind="ExternalInput").ap()
    # se: host-replicated, se[32*s+i, g, n] = inputs_se[core_base + g*4+s, n]
    se_d = nc.dram_tensor("se", [128, n_groups, NUM], F32, kind="ExternalInput").ap()
    # bias replicated 4x across partition groups: [128, 1]
    bias_d = nc.dram_tensor("bias", [128, 1], F32, kind="ExternalInput").ap()
    y_d = nc.dram_tensor("y", [b_local, O, H, W], F32, kind="ExternalOutput").ap()

    with tile.TileContext(nc) as tc:
        with (
            tc.tile_pool(name="xpool", bufs=2) as xpool,
            tc.tile_pool(name="wbpool", bufs=1) as wbpool,
            tc.tile_pool(name="wfin", bufs=2) as wfin_pool,
            tc.tile_pool(name="wtmp", bufs=3) as wtmp_pool,
            tc.tile_pool(name="sepool", bufs=1) as sepool,
            tc.tile_pool(name="outp", bufs=6) as outp,
            tc.tile_pool(name="psum", bufs=6, space="PSUM") as psump,
        ):
            # --- one-time loads -------------------------------------------
            wbsb = wbpool.tile([128, NUM, NTAPS, O], F32)
            for s in range(GROUP):
                nc.sync.dma_start(out=wbsb[32 * s : 32 * (s + 1)], in_=wb_d[:])
            se_sb = sepool.tile([128, n_groups, NUM], F32)
            nc.sync.dma_start(out=se_sb[:], in_=se_d[:])
            bias_sb = sepool.tile([128, 1], F32)
            nc.sync.dma_start(out=bias_sb[:], in_=bias_d[:])

            from contextlib import nullcontext

            rep_loop = tc.For_i(0, reps, 1) if reps > 1 else nullcontext()
            with rep_loop:
                _emit_body(nc, tc, n_groups, x_d, y_d, wbsb, se_sb, bias_sb,
                           xpool, wfin_pool, wtmp_pool, outp, psump,
                           abl_no_store, abl_no_xdma, abl_no_mm)

    if split_waits:
        _split_multiwait_insts(nc)
    return nc


def _emit_body(nc, tc, n_groups, x_d, y_d, wbsb, se_sb, bias_sb,
               xpool, wfin_pool, wtmp_pool, outp, psump,
               abl_no_store=False, abl_no_xdma=False, abl_no_mm=False):
    if True:
            for g in range(n_groups):
                # --- per-sample weight generation -------------------------
                # wfin[(s,i), t, o] = sum_n se[(s,i),g,n] * wbsb[(s,i),n,t,o]
                wfin = wfin_pool.tile([128, NTAPS, O], F32)
                tmps = []
                for n in range(1, NUM):
                    t_ = wtmp_pool.tile([128, NTAPS, O], F32, tag="wtmp")
                    nc.scalar.activation(
                        t_[:],
                        wbsb[:, n],
                        mybir.ActivationFunctionType.Identity,
                        scale=se_sb[:, g, n : n + 1],
                    )
                    tmps.append(t_)
                nc.scalar.activation(
                    wfin[:],
                    wbsb[:, 0],
                    mybir.ActivationFunctionType.Identity,
                    scale=se_sb[:, g, 0:1],
                )
                for t_ in tmps:
                    nc.vector.tensor_add(wfin[:], wfin[:], t_[:])
                # block-diagonal stationary: wbd[(s,i), t, 32s+o] = wfin,
                # zeros elsewhere (zeroed once per slot; diagonal blocks
                # overwritten each group). The f32->f32r copies satisfy the
                # fp32r rounding requirement.
                wbd = wfin_pool.tile([128, NTAPS, 128], F32R, tag="wbd")
                if g < 2:
                    nc.gpsimd.memset(wbd[:].bitcast(F32), 0.0)
                for s in range(GROUP):
                    nc.vector.tensor_copy(
                        wbd[32 * s : 32 * (s + 1), :, 32 * s : 32 * (s + 1)],
                        wfin[32 * s : 32 * (s + 1)],
                    )

                # --- load + pad group images (f32r = raw f32 bits) --------
                xt = xpool.tile([128, HP, WP], F32R)
                if g < 2:
                    nc.gpsimd.memset(xt[:, 0:1, :].bitcast(F32), 0.0)
                    nc.gpsimd.memset(xt[:, HP - 1 : HP, :].bitcast(F32), 0.0)
                    nc.gpsimd.memset(xt[:, 1 : HP - 1, 0:1].bitcast(F32), 0.0)
                    nc.gpsimd.memset(xt[:, 1 : HP - 1, WP - 1 : WP].bitcast(F32), 0.0)
                for s in range(GROUP):
                    if abl_no_xdma and g > 0:
                        continue
                    nc.sync.dma_start(
                        out=xt[32 * s : 32 * (s + 1), 1 : H + 1, 1 : W + 1],
                        in_=x_d[g * GROUP + s].bitcast(F32R),
                    )

                # --- conv: chunks of 4 output rows ------------------------
                for ci in range(N_CHUNKS):
                    h0 = ci * ROWS_PER_CHUNK
                    ps = psump.tile([128, ROWS_PER_CHUNK * W], F32)
                    if abl_no_mm:
                        nc.tensor.matmul(
                            ps[:], wbd[:, 0, :], xt[:, 0:ROWS_PER_CHUNK, 0:W],
                            start=True, stop=True)
                    else:
                        for tap in range(NTAPS):
                            kh, kw = divmod(tap, KK)
                            nc.tensor.matmul(
                                ps[:],
                                wbd[:, tap, :],
                                xt[:, h0 + kh : h0 + kh + ROWS_PER_CHUNK, kw : kw + W],
                                start=(tap == 0),
                                stop=(tap == NTAPS - 1),
                            )
                    if abl_no_store:
                        continue
                    ob = outp.tile([128, ROWS_PER_CHUNK * W], F32)
                    # alternate the PSUM-drain + store between ACT and DVE/SP
                    # so neither engine queue serializes the epilogue.
                    if ci % 2 == 0:
                        nc.scalar.activation(
                            ob[:],
                            ps[:],
                            mybir.ActivationFunctionType.Identity,
                            bias=bias_sb[:, 0:1],
                        )
                        store_eng = nc.scalar
                    else:
                        nc.vector.tensor_scalar_add(ob[:], ps[:], bias_sb[:, 0:1])
                        store_eng = nc.sync
                    # [128, 512] -> (s, o, 4 rows, W) in DRAM
                    store_eng.dma_start(
                        out=y_d[g * GROUP : (g + 1) * GROUP, :, h0 : h0 + ROWS_PER_CHUNK, :],
                        in_=ob[:],
                    )


def _host_prep(inputs, inputs_se, weight, bias):
    """Shard + relayout the inputs for the 8 per-core programs."""
    inputs = np.ascontiguousarray(inputs, dtype=np.float32)
    inputs_se = np.asarray(inputs_se, dtype=np.float32)
    weight = np.asarray(weight, dtype=np.float32)
    bias = np.asarray(bias, dtype=np.float32)

    # wb[i, n, t*32+o] = weight[o*288 + i*9 + t, n]
    wb = weight.reshape(O, C, NTAPS, NUM)          # [o, i, t, n]
    wb = np.ascontiguousarray(wb.transpose(1, 3, 2, 0).reshape(C, NUM, NTAPS * O))
    bias_rep = np.ascontiguousarray(np.tile(bias, GROUP)[:, None])  # [128, 1]

    in_maps = []
    for core in range(N_CORES):
        b0 = core * B_LOCAL
        se_loc = inputs_se[b0 : b0 + B_LOCAL]      # [16, 8]
        # se[32*s+i, g, n] = se_loc[g*4+s, n]
        se_exp = np.repeat(
            se_loc.reshape(N_GROUPS, GROUP, NUM).transpose(1, 0, 2), 32, axis=0
        )  # [4*32, g, n] with (s, i) partition order
        in_maps.append(
            {
                "x": inputs[b0 : b0 + B_LOCAL],
                "wb": wb,
                "se": np.ascontiguousarray(se_exp, dtype=np.float32),
                "bias": bias_rep,
            }
        )
    return in_maps


_NC_CACHE = {}


def kernel(inputs, inputs_se, weight, bias):
    if "nc" not in _NC_CACHE:
        _NC_CACHE["nc"] = build_program()
    nc = _NC_CACHE["nc"]
    in_maps = _host_prep(inputs, inputs_se, weight, bias)
    res = run_bass_kernel_spmd(nc, in_maps, list(range(N_CORES)))
    out = np.concatenate([res.results[i]["y"] for i in range(N_CORES)], axis=0)
    return out

